# revision 73
# baseline (speedup 1.0000x reference)
"""Trainium2 Bass kernel for nn_Attention_33157147525297.

Graph-mixed multi-head attention, B=64, N=196 tokens, D=768, H=12 heads.
Data-parallel over batch: 8 batches per NeuronCore x 8 cores.

Math restructuring (host side):
  reference: attn = softmax(G @ (q k^T * scale)); out = attn @ v
  G mixes the query index only, so G @ (q k^T) == (G q) k^T: the whole
  graph-mix collapses into a pre-mix of x on the query path: xg = G_s @ x,
  computed on device as a small GEMM.

Device pipeline per core (PSUM f32):
  mix:  xg^T = x[b]^T @ (16*G_s)^T, written as an fp8 hi half (ACT copy)
        plus an fp8 residual (DVE subtract) for the q GEMM.
  qkv:  all three projections run as compensated fp8e4 DoubleRow matmuls:
        W*64 = W8+dW8 and x = x8+dx8 (host-split, feature-major packed);
        out = W8@x8 + dW8@x8 + W8@dx8 - each DR instruction contracts
        2x128 rows at 0.5 cycles/row, so the three terms cost 0.75x the
        bf16 rows with BETTER accuracy (1.2e-3 vs 2.3e-3 per GEMM).
        Scale bookkeeping: k,q carry 64x (and q another 16x from the mix);
        exp() descales via its scale operand (1/65536); v carries 64x,
        folded into wp^T/64 on host.
  attn: per (b, head-pair): S^T for BOTH heads lands in one 2-bank psum
        tile (each matmul group bank-contained) so a single strided exp
        covers the pair; O~^T = [v_h|1]^T P^T - the ones column appended
        to v (65-column stride per head) makes the PV matmul emit the
        softmax sums as psum row 64 for free; sums -> bf16 sbuf row; K=1
        ones-matmuls broadcast them over 64 partitions; reciprocal
        psum->sbuf; DVE muls normalize into o^T (the odd head's write is
        partition-shifted to rows 64:128 - verified legal on walrus).
  proj: y = O @ Wp^T + bias over PACKED 128-token windows that cross
        batch boundaries (output [1568, 768] is contiguous in DRAM);
        o^T is split into 3 column-group tiles (b0 | b1-3 | b4-7) so the
        later groups' projections interleave into the attention window
        (dependency tracking is tile-granular).

Schedule: phases mix -> k (interleaved with the mix tail) -> q are
emitted as dense PE streams while DMAs land (both HWDGE queues, ordered
by first use; DMA transfers serialize on one DMA_ENGINES device in the
cost model and each dma_start costs ~600ns of serial queue issue, so
tensors are loaded with single multi-dim-AP DMAs and byte order
matters).  The attention window runs 48 (b, pair)
units in descending-batch order through a 3-stage software pipeline
(S+exp | PV+sums | bcast+recip+muls), with v chunks and unlocked proj
windows paced proportionally between units to keep the PE ahead of the
ACT-bound softmax chains.

Infra notes: this container's walrus accepts only ONE attached semaphore
wait per instruction - _install_wait_split() hoists extra waits onto
standalone EventSemaphore instructions.  Timing feedback comes from the
concourse cost-model TimelineSim (NTFF profiling hooks are unavailable
under this axon client).  TimelineSim: 165.8us (baseline 205.6us).
"""
import os
import sys
import numpy as np
import ml_dtypes

sys.path.insert(0, "/opt/trn_rl_repo")

SIZE, N_TOK, DIM, HEADS, HEAD_DIM, BATCH = 14, 196, 768, 12, 64, 64
N_CORES = 8
B_PER_CORE = BATCH // N_CORES  # 8
NT2 = 2 * N_TOK  # 392
NTB = N_TOK * B_PER_CORE  # 1568
BF16 = ml_dtypes.bfloat16

# token-dim partition tiles (196 = 128 + 68)
TOK_TILES = [(0, 128), (128, 68)]
# packed token windows for the projection (1568 = 12*128 + 32)
PROJ_WINDOWS = [(w * 128, 128) for w in range(12)] + [(1536, 32)]

LAST_EXEC_NS = None
LAST_TRACE = None


def _grid_g(factors):
    idx = np.arange(SIZE * SIZE).reshape(SIZE, SIZE)
    A = np.zeros((N_TOK, N_TOK), dtype=np.float32)
    for di, dj in [(-1, 0), (1, 0), (0, -1), (0, 1)]:
        for i in range(SIZE):
            for j in range(SIZE):
                ii, jj = i + di, j + dj
                if 0 <= ii < SIZE and 0 <= jj < SIZE:
                    A[idx[i, j], idx[ii, jj]] = 1.0
    NN = A / (A.sum(axis=1, keepdims=True) + 1.0)
    C = np.eye(N_TOK, dtype=np.float32) / 2.0
    return factors[0] * C + factors[1] * NN


def _install_wait_split():
    """This container's walrus rejects >1 attached semaphore wait per
    instruction ("Too many sync wait commands").  Hoist excess waits onto
    standalone InstEventSemaphore instructions just before, on the same
    engine - engine queues are in-order, so semantics are identical."""
    import concourse.mybir as mybir
    import concourse.tile as tile
    from concourse.vector_clock import ScopedClock

    TC = tile.TileContext
    if getattr(TC, "_wait_split_patched", False):
        return
    LIMIT = 1

    def _split(tc, inst):
        si = inst.sync_info
        if (si is None or not si.on_wait or len(si.on_wait) <= LIMIT
                or inst.engine == mybir.EngineType.Unassigned):
            return
        waits = list(si.on_wait)
        extra, keep = waits[:-LIMIT], waits[-LIMIT:]
        for i, w in enumerate(extra):
            ev = mybir.InstEventSemaphore(
                name=f"{inst.name}-ws{i}", engine=inst.engine,
                sync_info=mybir.SyncInfo(on_wait=[w], on_update=[]),
            )
            tc._add_instruction(ev)
        inst.sync_info = mybir.SyncInfo(on_wait=keep,
                                        on_update=list(si.on_update))

    orig_commit = TC._commit_instruction

    def patched_commit(self, inst, lazy_reg_writes=True):
        _split(self, inst)
        return orig_commit(self, inst, lazy_reg_writes=lazy_reg_writes)

    TC._commit_instruction = patched_commit

    def patched_drain_and_barrier(self, tick_clock, wait_clock):
        nc = self.nc
        probe = mybir.InstNoOp(
            name=f"drain-probe-{nc.next_id()}", engine=mybir.EngineType.SP)
        wait_clock.add_sem_waits(
            probe, ScopedClock({None: tick_clock.global_clock}))
        pw = probe.sync_info.on_wait if probe.sync_info else []
        for i, w in enumerate(pw):
            ev = mybir.InstEventSemaphore(
                name=f"drainw-{nc.next_id()}-{i}", engine=mybir.EngineType.SP,
                sync_info=mybir.SyncInfo(on_wait=[w], on_update=[]),
            )
            self._add_instruction(ev)
        nc.sync.drain()
        nc.all_engine_barrier()
        assert self.sems is not None
        popped = nc._tile_sem_poison_stack.pop()
        assert popped is self._sem_poison
        nc.clear_and_free_semaphores(list(self.sems.allocated().values()))
        nc.all_engine_barrier()

    TC._drain_and_barrier = patched_drain_and_barrier
    TC._wait_split_patched = True


SCHED = {
    "qk_alt": int(os.environ.get("K_QK_ALT", "1")),
    "sums_alt": int(os.environ.get("K_SUMS_ALT", "0")),
    "mix_inter": int(os.environ.get("K_MIX_INTER", "1")),
    "dram_bcast": int(os.environ.get("K_DRAM_BCAST", "0")),
}


def _build_bass():
    import concourse.bass as bass
    import concourse.mybir as mybir
    import concourse.tile as tile

    _install_wait_split()

    f32 = mybir.dt.float32
    bf16 = mybir.dt.bfloat16
    AF = mybir.ActivationFunctionType

    nc = bass.Bass()

    fp8 = mybir.dt.float8e4
    x_d = nc.declare_dram_parameter("x", [B_PER_CORE, N_TOK, DIM], bf16, isOutput=False)
    x8_d = nc.declare_dram_parameter("x8T", [6, 128, NTB], fp8, isOutput=False)
    dx8_d = nc.declare_dram_parameter("dx8T", [6, 128, NTB], fp8, isOutput=False)
    gT_d = nc.declare_dram_parameter("gT", [128, NT2], bf16, isOutput=False)
    wq_d = nc.declare_dram_parameter("wq8", [2, 128, 6 * DIM], fp8, isOutput=False)
    wk_d = nc.declare_dram_parameter("wk8", [2, 128, 6 * DIM], fp8, isOutput=False)
    wv_d = nc.declare_dram_parameter("wv8", [2, 128, 6 * DIM], fp8, isOutput=False)
    wp_d = nc.declare_dram_parameter("wpT", [128, 6 * DIM], bf16, isOutput=False)
    bias_d = nc.declare_dram_parameter("bias", [DIM], f32, isOutput=False)
    out_d = nc.declare_dram_parameter("out", [NTB, DIM], f32, isOutput=True)

    VW = 65  # per-head v columns: 64 v + 1 ones
    WS = 64.0       # fp8 weight scale (w ~0.02 -> ~1.3)
    XG_SCALE = 16.0  # fp8 xg scale
    S_SCALE = WS * WS * XG_SCALE  # k carries WS, q' carries WS*XG_SCALE

    from contextlib import ExitStack
    with tile.TileContext(nc) as tc, ExitStack() as _st:
        DBF = SCHED["dram_bcast"]
        const_p = _st.enter_context(tc.tile_pool(name="const", bufs=1))
        big_p = _st.enter_context(tc.tile_pool(name="big", bufs=1))
        cp_p = _st.enter_context(tc.tile_pool(name="cp", bufs=4))
        y_p = _st.enter_context(tc.tile_pool(name="yp", bufs=3))
        dr_p = _st.enter_context(tc.tile_pool(name="dr", bufs=8, space="DRAM"))
        ps_big = _st.enter_context(
            tc.tile_pool(name="ps_big", bufs=3, space="PSUM"))
        ps_s = _st.enter_context(
            tc.tile_pool(name="ps_s", bufs=1, space="PSUM"))
        ps_pv = _st.enter_context(
            tc.tile_pool(name="ps_pv", bufs=int(os.environ.get("K_PV", "2")), space="PSUM"))
        ps_bc = None if DBF else _st.enter_context(
            tc.tile_pool(name="ps_bc", bufs=1, space="PSUM"))
        if True:
            # ---- DMAs: SP queue feeds the mix (gT, packed x) and late
            # weights; ACT queue feeds the k path (wk, xT).  Each HWDGE
            # queue issues serially (~600ns per DMA) so order = first use.
            def load_w(d, nm, eng):
                t = const_p.tile([128, 6 * DIM], bf16, name=nm)
                eng.dma_start(out=t, in_=d[:, :])
                return t

            # mix inputs first, paired across both HWDGE queues (gT+x feed
            # the mix, the only PE work available in the first ~10us);
            # k-path (wk + xT halves) next; the rest in order of first use.
            # token-tile halves loaded separately: no padding bytes.
            xt0 = [const_p.tile([128, 2 * DIM], bf16, name=f"xt0_{bp}")
                   for bp in range(4)]
            xt1 = const_p.tile([128, B_PER_CORE * DIM], bf16, name="xt1")
            xt1v = xt1.rearrange("p (b c) -> p b c", b=B_PER_CORE)
            g_pk = const_p.tile([128, NT2], bf16, name="g_pk")
            nc.scalar.dma_start(out=g_pk, in_=gT_d[:, :])
            nc.sync.dma_start(
                out=xt1v[0:68],
                in_=x_d[:, 128:196, :].rearrange("b p c -> p b c"))
            for bp in range(4):
                (nc.scalar if bp % 2 == 0 else nc.sync).dma_start(
                    out=xt0[bp].rearrange("p (b c) -> p b c", b=2),
                    in_=x_d[2 * bp:2 * bp + 2, 0:128, :]
                    .rearrange("b p c -> p b c"))

            def load_w8(d, nm, eng):
                # [2, 128, 6*DIM]: hi/lo fp8 halves of a 64x-scaled weight
                t = const_p.tile([128, 2 * 6 * DIM], fp8, name=nm)
                eng.dma_start(out=t.rearrange("p (h c) -> p h c", h=2),
                              in_=d[:, :, :].rearrange("h p c -> p h c"))
                return t

            # x8/dx8: [128, kt, NTB] fp8 (DoubleRow pairs slice dim 1)
            x8_sb = big_p.tile([128, 6 * NTB], fp8, name="x8")
            dx8_sb = big_p.tile([128, 6 * NTB], fp8, name="dx8")
            x8v = x8_sb.rearrange("p (k c) -> p k c", k=6)
            dx8v = dx8_sb.rearrange("p (k c) -> p k c", k=6)
            nc.scalar.dma_start(
                out=x8v, in_=x8_d[:, :, :].rearrange("k p c -> p k c"))
            nc.sync.dma_start(
                out=dx8v, in_=dx8_d[:, :, :].rearrange("k p c -> p k c"))
            wk_sb = load_w8(wk_d, "wk", nc.sync)
            wq_sb = load_w8(wq_d, "wq", nc.scalar)
            wv_sb = load_w8(wv_d, "wv", nc.sync)
            wp_sb = load_w(wp_d, "wp", nc.sync)
            bias_sb = const_p.tile([128, DIM], f32, name="bias")
            nc.sync.dma_start(out=bias_sb,
                              in_=bias_d[None, :].broadcast_to([128, DIM]))

            def wsl(w, kt, c0, c1):
                return w[:, kt * DIM + c0:kt * DIM + c1]

            ones64 = const_p.tile([128, 64], bf16, name="ones64")
            nc.vector.memset(ones64, 1.0)

            def warmup(n):
                # dependency-free matmuls: keep the PE clock ramp warm while
                # the first DMAs land (output never read)
                for i in range(n):
                    ps = ps_bc.tile([128, NT2], f32, tag="bc2", name="wu")
                    nc.tensor.matmul(ps[0:64, :], ones64, ones64[:, 0:1]
                                     .broadcast_to([128, NT2]) if False else
                                     ones64[:, 0:1], start=True, stop=True)

            xg8_sb = big_p.tile([128, 6 * NTB], fp8, name="xg8")
            dxg8_sb = big_p.tile([128, 6 * NTB], fp8, name="dxg8")
            xg8v = xg8_sb.rearrange("p (k c) -> p k c", k=6)
            dxg8v = dxg8_sb.rearrange("p (k c) -> p k c", k=6)
            qT_sb = [big_p.tile([128, NTB], bf16, name=f"qT{k}")
                     for k in range(6)]
            kT_sb = [big_p.tile([128, NTB], bf16, name=f"kT{k}")
                     for k in range(6)]
            v_sb = [
                [big_p.tile([128, HEADS * VW], bf16, name=f"v{b}_{ti}")
                 for ti in range(2)]
                for b in range(B_PER_CORE)
            ]
            for b in range(B_PER_CORE):
                for ti in range(2):
                    ones_cols = v_sb[b][ti].rearrange(
                        "p (h e) -> p h e", e=VW)[:, :, 64:65]
                    nc.vector.memset(ones_cols, 1.0)

            # o^T in 3 groups: batches 4-7 / 1-3 / 0.  Attention runs b7
            # first, so group C (b4-7) unblocks its projection mid-pipeline
            # and the tail is only batch 0's 2 windows (deps are
            # tile-granular; a proj stationary AP cannot span group tiles).
            _ogv = 3
            OG = {5: [(0, 1), (1, 2), (2, 4), (4, 6), (6, 8)],
                  4: [(0, 1), (1, 2), (2, 4), (4, 8)],
                  3: [(0, 1), (1, 4), (4, 8)],
                  2: [(0, 4), (4, 8)],
                  6: [(0, 2), (2, 4), (4, 8)]}[_ogv]  # group -> batches
            o_sb = [[big_p.tile([128, (hi - lo) * N_TOK], bf16,
                                name=f"o{k}_{g}")
                     for g, (lo, hi) in enumerate(OG)] for k in range(6)]

            def ogrp(b):
                for g, (lo, hi) in enumerate(OG):
                    if lo <= b < hi:
                        return g, (b - lo) * N_TOK
                raise AssertionError

            # proj windows per group: (g, local w0, wsz); global 128-aligned
            PW = []
            for g, (lo, hi) in enumerate(OG):
                t0g, t1g = lo * N_TOK, hi * N_TOK
                bounds = [t0g] + [t for t in range(0, NTB, 128)
                                  if t0g < t < t1g] + [t1g]
                for a, bnd in zip(bounds, bounds[1:]):
                    PW.append((g, a - t0g, bnd - a))

            # ---- emission helpers ----
            MIX_POOLS = None  # set after pools exist

            def mix_chunk(bp, mt, _eng=None):
                # two batches (2bp, 2bp+1) per psum bank; round-robin all
                # four psum pools so copy latency never gates the PE
                pool, tag = MIX_POOLS[(bp * 6 + mt) % len(MIX_POOLS)]
                ps = pool.tile([128, NT2], f32, tag=tag, name="psm")
                for i in range(2):
                    b = 2 * bp + i
                    for ti, (t0, tsz) in enumerate(TOK_TILES):
                        if ti == 0:
                            st = xt0[bp][:tsz, i * DIM + mt * 128:
                                         i * DIM + (mt + 1) * 128]
                        else:
                            st = xt1v[:tsz, b, mt * 128:(mt + 1) * 128]
                        nc.tensor.matmul(
                            ps[:, i * N_TOK:(i + 1) * N_TOK], st,
                            g_pk[:tsz, ti * N_TOK:(ti + 1) * N_TOK],
                            start=(ti == 0), stop=(ti == 1),
                        )
                cs = slice(bp * NT2, (bp + 1) * NT2)
                nc.scalar.activation(xg8v[:, mt, cs], ps, AF.Copy)
                nc.vector.tensor_sub(dxg8v[:, mt, cs], ps, xg8v[:, mt, cs])

            DR = mybir.MatmulPerfMode.DoubleRow

            def qk_chunk(which, mt, nt):
                if which == "q":
                    dst, w, xhi, xlo = qT_sb, wq_sb, xg8v, dxg8v
                    cbase = nt * NT2
                else:
                    dst, w = kT_sb, wk_sb
                    xhi, xlo = x8v, dx8v
                    cbase = nt * NT2
                wv_ = w.rearrange("p (h k m) -> p h k m", h=2, k=6)
                ps = ps_big.tile([128, NT2], f32, tag="psA", name="ps")
                ms = slice(mt * 128, (mt + 1) * 128)
                nmm = 0
                for half in range(2):
                    c0 = cbase + half * N_TOK
                    cs = slice(c0, c0 + N_TOK)
                    for t in range(3):
                        kp = slice(2 * t, 2 * t + 2)
                        for whalf, xsrc in ((0, xhi), (1, xhi), (0, xlo)):
                            nc.tensor.matmul(
                                ps[:, half * N_TOK:(half + 1) * N_TOK],
                                wv_[:, whalf, kp, ms], xsrc[:, kp, cs],
                                start=(nmm == 0), stop=(nmm == 17),
                                perf_mode=DR,
                            )
                            nmm += 1
                par = 0
                sel = ((mt + nt) % 2 if par == 0 else
                       (mt % 2 if par == 1 else nt % 2))
                if not SCHED["qk_alt"] or sel:
                    nc.scalar.activation(
                        dst[mt][:, nt * NT2:(nt + 1) * NT2], ps, AF.Copy)
                else:
                    nc.vector.tensor_copy(
                        dst[mt][:, nt * NT2:(nt + 1) * NT2], ps)

            def v_chunk(b, ti, nt):
                t0, tsz = TOK_TILES[ti]
                veng = 0
                eng = (b * 4 + ti * 2 + nt) % 2 if veng == 0 else veng - 1
                wvv = wv_sb.rearrange("p (h k m) -> p h k m", h=2, k=6)
                ps = ps_big.tile([128, NT2], f32, tag="psA", name="ps")
                xa, dxa = x8v, dx8v
                xs = slice(b * N_TOK + t0, b * N_TOK + t0 + tsz)
                nmm = 0
                for half in range(2):
                    m0 = nt * 384 + half * 192
                    ms = slice(m0, m0 + 192)
                    for t in range(3):
                        kp = slice(2 * t, 2 * t + 2)
                        for whalf, xsrc in ((0, xa), (0, dxa), (1, xa)):
                            nc.tensor.matmul(
                                ps[:tsz, half * 192:half * 192 + 192],
                                xsrc[:, kp, xs], wvv[:, whalf, kp, ms],
                                start=(nmm == 0), stop=(nmm == 17),
                                perf_mode=DR,
                            )
                            nmm += 1
                dst = v_sb[b][ti].rearrange(
                    "p (h e) -> p h e", e=VW)[:tsz, 6 * nt:6 * nt + 6, 0:64]
                src = ps[:tsz, :384].rearrange("p (h e) -> p h e", e=64)
                if eng:
                    nc.scalar.activation(dst, src, AF.Copy)
                else:
                    nc.vector.tensor_copy(dst, src)

            ystash = {}

            def proj_chunk(g, w0, wsz, nt):
                ps = ps_big.tile([128, NT2], f32, tag="psA", name="ps")
                for kt in range(6):
                    nc.tensor.matmul(
                        ps[:wsz, :384],
                        o_sb[kt][g][:, w0:w0 + wsz],
                        wsl(wp_sb, kt, nt * 384, (nt + 1) * 384),
                        start=(kt == 0), stop=(kt == 5),
                    )
                if nt == 0:
                    y_sb = y_p.tile([128, DIM], f32, tag="y", name="y_sb")
                    ystash[(g, w0)] = y_sb
                else:
                    y_sb = ystash.pop((g, w0))
                nc.vector.tensor_add(
                    y_sb[:wsz, nt * 384:(nt + 1) * 384], ps[:wsz, :384],
                    bias_sb[:wsz, nt * 384:(nt + 1) * 384])
                if nt == 1:
                    a0 = OG[g][0] * N_TOK + w0
                    nc.sync.dma_start(out=out_d[a0:a0 + wsz, :],
                                      in_=y_sb[:wsz])

            # ---- attention stages (3-stage software pipeline) ----
            stash = {}

            def attn_s_exp(b, p):
                # S for BOTH heads in one 2-bank psum tile (cols 0:392 and
                # 512:904 - each matmul group stays inside one bank), then a
                # single strided exp covers both.
                c0 = b * N_TOK
                s_ps = ps_s.tile([128, 1024], f32, tag="s", name="s")
                for hi in range(2):
                    hb = hi * 64
                    for ti, (t0, tsz) in enumerate(TOK_TILES):
                        nc.tensor.matmul(
                            s_ps[:tsz,
                                 hi * 512 + ti * N_TOK:
                                 hi * 512 + (ti + 1) * N_TOK],
                            kT_sb[p][hb:hb + 64, c0 + t0:c0 + t0 + tsz],
                            qT_sb[p][hb:hb + 64, c0:c0 + N_TOK],
                            start=True, stop=True,
                        )
                pT = cp_p.tile([128, 2 * NT2], bf16, tag="pT", name="pT")
                sv = s_ps.rearrange("q (h c) -> q h c", h=2)[:, :, 0:NT2]
                pv_ = pT.rearrange("q (h c) -> q h c", h=2)
                nc.scalar.activation(pv_, sv, AF.Exp, scale=1.0 / S_SCALE)
                stash[(b, p, "pT")] = pT

            def attn_pv(b, p):
                pT = stash.pop((b, p, "pT"))
                pv = ps_pv.tile([128, NT2], f32, tag="pv", name="pv")
                for hi in range(2):
                    h = 2 * p + hi
                    for ti, (t0, tsz) in enumerate(TOK_TILES):
                        nc.tensor.matmul(
                            pv[0:VW, hi * N_TOK:(hi + 1) * N_TOK],
                            v_sb[b][ti][:tsz, h * VW:(h + 1) * VW],
                            pT[:tsz,
                               hi * NT2 + ti * N_TOK:
                               hi * NT2 + (ti + 1) * N_TOK],
                            start=(ti == 0), stop=(ti == 1),
                        )
                if SCHED["dram_bcast"]:
                    stash[(b, p, "pv")] = (pv, None)
                    return
                if 0:
                    stash[(b, p, "pv")] = (pv, None)
                    return
                srow = cp_p.tile([128, NT2], bf16, tag="srow", name="srow")
                if not SCHED["sums_alt"] or (b + p) % 2:
                    nc.scalar.activation(srow[64:65, :], pv[64:65, :], AF.Copy)
                else:
                    nc.vector.tensor_copy(srow[64:65, :], pv[64:65, :])
                stash[(b, p, "pv")] = (pv, srow)

            def attn_norm_a(b, p):
                # reciprocal of the sums row straight from psum, spill to DRAM
                pv, _ = stash[(b, p, "pv")]
                rrow = cp_p.tile([128, NT2], bf16, tag="rrow", name="rrow")
                with nc.allow_low_precision(reason="softmax recip bf16"):
                    nc.vector.reciprocal(rrow[64:65, :], pv[64:65, :])
                d = dr_p.tile([1, NT2], bf16, name="rd")
                nc.sync.dma_start(out=d, in_=rrow[64:65, :])
                stash[(b, p, "rd")] = d

            def attn_norm_b(b, p):
                # DRAM -> SBUF partition-broadcast of the reciprocal row
                d = stash.pop((b, p, "rd"))
                rbc = cp_p.tile([128, NT2], bf16, tag="rbc", name="rbc")
                nc.sync.dma_start(
                    out=rbc[0:64, :],
                    in_=d[0, :][None, :].broadcast_to([64, NT2]))
                stash[(b, p, "rbc")] = rbc

            def attn_norm_c(b, p):
                g, c0 = ogrp(b)
                pv, _ = stash.pop((b, p, "pv"))
                rbc = stash.pop((b, p, "rbc"))
                nc.vector.tensor_mul(o_sb[p][g][0:64, c0:c0 + N_TOK],
                                     pv[0:64, 0:N_TOK], rbc[0:64, 0:N_TOK])
                nc.vector.tensor_mul(o_sb[p][g][64:128, c0:c0 + N_TOK],
                                     pv[0:64, N_TOK:NT2], rbc[0:64, N_TOK:NT2])

            def attn_norm(b, p):
                g, c0 = ogrp(b)
                pv, srow = stash.pop((b, p, "pv"))
                if srow is None:
                    srow = cp_p.tile([128, NT2], bf16, tag="srow", name="srow")
                    nc.scalar.activation(srow[64:65, :], pv[64:65, :], AF.Copy)
                bc = ps_bc.tile([128, NT2], f32, tag="bc2", name="bc")
                nc.tensor.matmul(
                    bc[0:64, :], ones64[64:65, :], srow[64:65, :],
                    start=True, stop=True, tile_position=(64, 0),
                )
                rcp = cp_p.tile([128, NT2], bf16, tag="rcp", name="rcp")
                with nc.allow_low_precision(reason="softmax recip bf16"):
                    nc.vector.reciprocal(rcp[0:64, :], bc[0:64, :])
                nc.vector.tensor_mul(o_sb[p][g][0:64, c0:c0 + N_TOK],
                                     pv[0:64, 0:N_TOK], rcp[0:64, 0:N_TOK])
                nc.vector.tensor_mul(o_sb[p][g][64:128, c0:c0 + N_TOK],
                                     pv[0:64, N_TOK:NT2], rcp[0:64, N_TOK:NT2])

            # ---- schedule ----
            MIX_POOLS = [(ps_s, "s"), (ps_pv, "pv"), (ps_big, "psA")]
            if ps_bc is not None:
                MIX_POOLS.insert(0, (ps_bc, "bc2"))
            # mix bp0-1 first (x0-x3 land early), then k 1:1 with the
            # remaining mix chunks (independent), then q.
            mixes = [(bp, mt) for bp in range(4) for mt in range(6)]
            nup = 24
            for bp, mt in mixes[:nup]:
                mix_chunk(bp, mt, "")
            mix_rest = mixes[nup:]
            if 0:
                korder = [(mt, nt) for mt in range(6) for nt in range(4)]
            else:
                korder = [(mt, nt) for nt in range(4) for mt in range(6)]
            for i, (mt, nt) in enumerate(korder):
                qk_chunk("k", mt, nt)
                if i < len(mix_rest):
                    mix_chunk(*mix_rest[i], "")
            if int(os.environ.get("K_QORD", "0")):
                for mt in range(6):
                    for nt in range(4):
                        qk_chunk("q", mt, nt)
            else:
                for nt in range(4):
                    for mt in range(6):
                        qk_chunk("q", mt, nt)

            # attention-window dense stream: v batches 7..0, proj on
            # group eligibility
            dense = []
            vpos = {}
            vlate = []
            for vb in range(7, -1, -1):
                tgt = dense if vb >= int(os.environ.get("K_VHOLD", "3")) else vlate
                for ti in range(2):
                    for nt in range(2):
                        tgt.append(("v", vb, ti, nt))
                vpos[vb] = (len(dense) - 1 if vb >= 2
                            else 24 + 2 + len(vlate) + 2)

            # attention unit order: descending batch; each o-group's proj
            # unlocks when its lowest batch (last in this order) is normed
            units = [(b, p) for b in range(7, -1, -1) for p in range(6)]

            MARGIN = int(os.environ.get("K_MARGIN", "0"))
            proj_after = {}  # last batch of group (in unit order) -> chunks
            for g, (lo, hi) in enumerate(OG):
                gws = [pw for pw in PW if pw[0] == g]
                if 1:
                    gws = gws[::-1]
                proj_after[lo] = [  # b7-first => smallest batch finishes last
                    ("p",) + w + (nt,)
                    for w in gws
                    for nt in range(2)]

            emitted = 0
            norm_count = {b: 0 for b in range(B_PER_CORE)}

            def emit_dense(n):
                nonlocal_emitted = 0
                while nonlocal_emitted < n and dense:
                    c = dense.pop(0)
                    if c[0] == "v":
                        v_chunk(*c[1:])
                    elif c[0] in ("k", "q"):
                        qk_chunk(c[0], c[1], c[2])
                    else:
                        proj_chunk(*c[1:])
                    nonlocal_emitted += 1
                return nonlocal_emitted

            def unit_ready(i):
                b, p = units[i]
                if b < int(os.environ.get("K_VHOLD", "3")):
                    # late-released v: gated by the release point
                    return i >= 27 and emitted >= vpos[b]
                return emitted >= vpos[b] + MARGIN

            i = 0
            n_units = len(units)
            credit = 0.0
            # dense chunks that will EVER be available during attention:
            total_dense_all = len(dense) + len(vlate) + sum(
                len(v) for v in proj_after.values())
            pace = total_dense_all / float(n_units) * float(os.environ.get("K_PACE", "1.0"))
            DB = SCHED["dram_bcast"]
            n_stages = 5 if DB else 3
            while i < n_units + n_stages - 1:
                if i < n_units and not unit_ready(i):
                    got = emit_dense(1)
                    emitted += got
                    if got == 0:
                        # dense exhausted but unit not "ready": emit anyway
                        pass
                    else:
                        continue
                if i < n_units:
                    attn_s_exp(*units[i])
                if 1 <= i < n_units + 1:
                    attn_pv(*units[i - 1])
                if i == 22 and vlate:
                    dense.extend(vlate)
                    vlate = []
                if DB:
                    if 2 <= i < n_units + 2:
                        attn_norm_a(*units[i - 2])
                    if 3 <= i < n_units + 3:
                        attn_norm_b(*units[i - 3])
                    if 4 <= i:
                        b, p = units[i - 4]
                        attn_norm_c(b, p)
                        norm_count[b] += 1
                        if norm_count[b] == 6 and b in proj_after:
                            dense.extend(proj_after.pop(b))
                elif 2 <= i:
                    b, p = units[i - 2]
                    attn_norm(b, p)
                    norm_count[b] += 1
                    if norm_count[b] == 6 and b in proj_after:
                        dense.extend(proj_after.pop(b))
                credit += pace
                take = int(credit)
                credit -= take
                emitted += emit_dense(take)
                i += 1
            while dense:
                emitted += emit_dense(1)

    return nc


_CACHED_NC = None


def kernel(x, w_qkv, w_proj, b_proj, factors):
    global LAST_EXEC_NS, LAST_TRACE, _CACHED_NC
    from concourse.bass_utils import run_bass_kernel_spmd

    E4M3 = ml_dtypes.float8_e4m3fn
    WS, XG_SCALE = 64.0, 16.0
    factors = np.asarray(factors, dtype=np.float32)
    scale = HEAD_DIM ** -0.5
    G_s = _grid_g(factors) * scale
    gT_f = G_s.T * XG_SCALE  # [196, 196], fp8-range scaled
    gT = np.zeros((128, NT2), dtype=np.float32)
    gT[:, 0:N_TOK] = gT_f[0:128]
    gT[0:68, N_TOK:NT2] = gT_f[128:196]
    gT = gT.astype(BF16)

    w_qkv = np.asarray(w_qkv, dtype=np.float32)

    def il(wT, d=BF16):
        # [768, 768] feature-major -> [128, 6*768] kt-interleaved
        return np.ascontiguousarray(
            wT.reshape(6, 128, DIM).transpose(1, 0, 2).reshape(128, 6 * DIM)
        ).astype(d)

    def split8(wT):
        # 64x-scaled fp8 hi/lo halves, kt-interleaved: [2, 128, 6*DIM]
        ws = il(wT * WS, np.float32)
        hi = ws.astype(E4M3)
        lo = (ws - hi.astype(np.float32)).astype(E4M3)
        return np.stack([hi, lo])

    in_common = {
        "gT": gT,
        "wq8": split8(w_qkv[0:DIM, :].T),
        "wk8": split8(w_qkv[DIM:2 * DIM, :].T),
        "wv8": split8(w_qkv[2 * DIM:3 * DIM, :].T),
        "wpT": il(np.asarray(w_proj, dtype=np.float32).T / WS),
        "bias": np.asarray(b_proj, dtype=np.float32),
    }
    x = np.asarray(x, dtype=np.float32).astype(BF16)
    in_maps = []
    for c in range(N_CORES):
        xc = x[c * B_PER_CORE:(c + 1) * B_PER_CORE]  # [8, 196, 768]
        xT = np.ascontiguousarray(
            xc.transpose(2, 0, 1).reshape(6, 128, NTB)).astype(np.float32)
        x8 = xT.astype(E4M3)
        dx8 = (xT - x8.astype(np.float32)).astype(E4M3)
        in_maps.append({"x": np.ascontiguousarray(xc), "x8T": x8,
                        "dx8T": dx8, **in_common})

    if _CACHED_NC is None:
        _CACHED_NC = _build_bass()
    nc = _CACHED_NC

    trace = bool(int(os.environ.get("KERNEL_TRACE", "0")))
    res = run_bass_kernel_spmd(nc, in_maps, core_ids=list(range(N_CORES)),
                               trace=trace)
    LAST_EXEC_NS = res.exec_time_ns
    if res.instructions_and_trace is not None:
        LAST_TRACE = res.instructions_and_trace[1]
    out = np.concatenate(
        [res.results[c]["out"].reshape(B_PER_CORE, N_TOK, DIM)
         for c in range(N_CORES)], axis=0)
    return out.astype(np.float32)


# revision 75
# speedup vs baseline: 1.0031x; 1.0031x over previous
"""Trainium2 Bass kernel for nn_Attention_33157147525297.

Graph-mixed multi-head attention, B=64, N=196 tokens, D=768, H=12 heads.
Data-parallel over batch: 8 batches per NeuronCore x 8 cores.

Math restructuring (host side):
  reference: attn = softmax(G @ (q k^T * scale)); out = attn @ v
  G mixes the query index only, so G @ (q k^T) == (G q) k^T: the whole
  graph-mix collapses into a pre-mix of x on the query path: xg = G_s @ x,
  computed on device as a small GEMM.

Device pipeline per core (PSUM f32):
  mix:  xg^T = x[b]^T @ (16*G_s)^T, written as an fp8 hi half (ACT copy)
        plus an fp8 residual (DVE subtract) for the q GEMM.
  qkv:  all three projections run as compensated fp8e4 DoubleRow matmuls:
        W*64 = W8+dW8 and x = x8+dx8 (host-split, feature-major packed);
        out = W8@x8 + dW8@x8 + W8@dx8 - each DR instruction contracts
        2x128 rows at 0.5 cycles/row, so the three terms cost 0.75x the
        bf16 rows with BETTER accuracy (1.2e-3 vs 2.3e-3 per GEMM).
        Scale bookkeeping: k,q carry 64x (and q another 16x from the mix);
        exp() descales via its scale operand (1/65536); v carries 64x,
        folded into wp^T/64 on host.
  attn: per (b, head-pair): S^T for BOTH heads lands in one 2-bank psum
        tile (each matmul group bank-contained) so a single strided exp
        covers the pair; O~^T = [v_h|1]^T P^T - the ones column appended
        to v (65-column stride per head) makes the PV matmul emit the
        softmax sums as psum row 64 for free; sums -> bf16 sbuf row; K=1
        ones-matmuls broadcast them over 64 partitions; reciprocal
        psum->sbuf; DVE muls normalize into o^T (the odd head's write is
        partition-shifted to rows 64:128 - verified legal on walrus).
  proj: y = O @ Wp^T + bias over PACKED 128-token windows that cross
        batch boundaries (output [1568, 768] is contiguous in DRAM);
        o^T is split into 3 column-group tiles (b0 | b1-3 | b4-7) so the
        later groups' projections interleave into the attention window
        (dependency tracking is tile-granular).

Schedule: phases mix -> k (interleaved with the mix tail) -> q are
emitted as dense PE streams while DMAs land (both HWDGE queues, ordered
by first use; DMA transfers serialize on one DMA_ENGINES device in the
cost model and each dma_start costs ~600ns of serial queue issue, so
tensors are loaded with single multi-dim-AP DMAs and byte order
matters).  The attention window runs 48 (b, pair)
units in descending-batch order through a 3-stage software pipeline
(S+exp | PV+sums | bcast+recip+muls), with v chunks and unlocked proj
windows paced proportionally between units to keep the PE ahead of the
ACT-bound softmax chains.

Infra notes: this container's walrus accepts only ONE attached semaphore
wait per instruction - _install_wait_split() hoists extra waits onto
standalone EventSemaphore instructions.  Timing feedback comes from the
concourse cost-model TimelineSim (NTFF profiling hooks are unavailable
under this axon client).  TimelineSim: 165.8us (baseline 205.6us).
"""
import os
import sys
import numpy as np
import ml_dtypes

sys.path.insert(0, "/opt/trn_rl_repo")

SIZE, N_TOK, DIM, HEADS, HEAD_DIM, BATCH = 14, 196, 768, 12, 64, 64
N_CORES = 8
B_PER_CORE = BATCH // N_CORES  # 8
NT2 = 2 * N_TOK  # 392
NTB = N_TOK * B_PER_CORE  # 1568
BF16 = ml_dtypes.bfloat16

# token-dim partition tiles (196 = 128 + 68)
TOK_TILES = [(0, 128), (128, 68)]
# packed token windows for the projection (1568 = 12*128 + 32)
PROJ_WINDOWS = [(w * 128, 128) for w in range(12)] + [(1536, 32)]

LAST_EXEC_NS = None
LAST_TRACE = None


def _grid_g(factors):
    idx = np.arange(SIZE * SIZE).reshape(SIZE, SIZE)
    A = np.zeros((N_TOK, N_TOK), dtype=np.float32)
    for di, dj in [(-1, 0), (1, 0), (0, -1), (0, 1)]:
        for i in range(SIZE):
            for j in range(SIZE):
                ii, jj = i + di, j + dj
                if 0 <= ii < SIZE and 0 <= jj < SIZE:
                    A[idx[i, j], idx[ii, jj]] = 1.0
    NN = A / (A.sum(axis=1, keepdims=True) + 1.0)
    C = np.eye(N_TOK, dtype=np.float32) / 2.0
    return factors[0] * C + factors[1] * NN


def _install_wait_split():
    """This container's walrus rejects >1 attached semaphore wait per
    instruction ("Too many sync wait commands").  Hoist excess waits onto
    standalone InstEventSemaphore instructions just before, on the same
    engine - engine queues are in-order, so semantics are identical."""
    import concourse.mybir as mybir
    import concourse.tile as tile
    from concourse.vector_clock import ScopedClock

    TC = tile.TileContext
    if getattr(TC, "_wait_split_patched", False):
        return
    LIMIT = 1

    def _split(tc, inst):
        si = inst.sync_info
        if (si is None or not si.on_wait or len(si.on_wait) <= LIMIT
                or inst.engine == mybir.EngineType.Unassigned):
            return
        waits = list(si.on_wait)
        extra, keep = waits[:-LIMIT], waits[-LIMIT:]
        for i, w in enumerate(extra):
            ev = mybir.InstEventSemaphore(
                name=f"{inst.name}-ws{i}", engine=inst.engine,
                sync_info=mybir.SyncInfo(on_wait=[w], on_update=[]),
            )
            tc._add_instruction(ev)
        inst.sync_info = mybir.SyncInfo(on_wait=keep,
                                        on_update=list(si.on_update))

    orig_commit = TC._commit_instruction

    def patched_commit(self, inst, lazy_reg_writes=True):
        _split(self, inst)
        return orig_commit(self, inst, lazy_reg_writes=lazy_reg_writes)

    TC._commit_instruction = patched_commit

    def patched_drain_and_barrier(self, tick_clock, wait_clock):
        nc = self.nc
        probe = mybir.InstNoOp(
            name=f"drain-probe-{nc.next_id()}", engine=mybir.EngineType.SP)
        wait_clock.add_sem_waits(
            probe, ScopedClock({None: tick_clock.global_clock}))
        pw = probe.sync_info.on_wait if probe.sync_info else []
        for i, w in enumerate(pw):
            ev = mybir.InstEventSemaphore(
                name=f"drainw-{nc.next_id()}-{i}", engine=mybir.EngineType.SP,
                sync_info=mybir.SyncInfo(on_wait=[w], on_update=[]),
            )
            self._add_instruction(ev)
        nc.sync.drain()
        nc.all_engine_barrier()
        assert self.sems is not None
        popped = nc._tile_sem_poison_stack.pop()
        assert popped is self._sem_poison
        nc.clear_and_free_semaphores(list(self.sems.allocated().values()))
        nc.all_engine_barrier()

    TC._drain_and_barrier = patched_drain_and_barrier
    TC._wait_split_patched = True


SCHED = {
    "qk_alt": int(os.environ.get("K_QK_ALT", "1")),
    "sums_alt": int(os.environ.get("K_SUMS_ALT", "0")),
    "mix_inter": int(os.environ.get("K_MIX_INTER", "1")),
    "dram_bcast": int(os.environ.get("K_DRAM_BCAST", "0")),
}


def _build_bass():
    import concourse.bass as bass
    import concourse.mybir as mybir
    import concourse.tile as tile

    _install_wait_split()

    f32 = mybir.dt.float32
    bf16 = mybir.dt.bfloat16
    AF = mybir.ActivationFunctionType

    nc = bass.Bass()

    fp8 = mybir.dt.float8e4
    x_d = nc.declare_dram_parameter("x", [B_PER_CORE, N_TOK, DIM], bf16, isOutput=False)
    x8_d = nc.declare_dram_parameter("x8T", [6, 128, NTB], fp8, isOutput=False)
    dx8_d = nc.declare_dram_parameter("dx8T", [6, 128, NTB], fp8, isOutput=False)
    gT_d = nc.declare_dram_parameter("gT", [128, NT2], bf16, isOutput=False)
    wq_d = nc.declare_dram_parameter("wq8", [2, 128, 6 * DIM], fp8, isOutput=False)
    wk_d = nc.declare_dram_parameter("wk8", [2, 128, 6 * DIM], fp8, isOutput=False)
    wv_d = nc.declare_dram_parameter("wv8", [2, 128, 6 * DIM], fp8, isOutput=False)
    wp_d = nc.declare_dram_parameter("wpT", [128, 6 * DIM], bf16, isOutput=False)
    bias_d = nc.declare_dram_parameter("bias", [DIM], f32, isOutput=False)
    out_d = nc.declare_dram_parameter("out", [NTB, DIM], f32, isOutput=True)

    VW = 65  # per-head v columns: 64 v + 1 ones
    WS = 64.0       # fp8 weight scale (w ~0.02 -> ~1.3)
    XG_SCALE = 16.0  # fp8 xg scale
    S_SCALE = WS * WS * XG_SCALE  # k carries WS, q' carries WS*XG_SCALE

    from contextlib import ExitStack
    with tile.TileContext(nc) as tc, ExitStack() as _st:
        DBF = SCHED["dram_bcast"]
        const_p = _st.enter_context(tc.tile_pool(name="const", bufs=1))
        big_p = _st.enter_context(tc.tile_pool(name="big", bufs=1))
        cp_p = _st.enter_context(tc.tile_pool(name="cp", bufs=4))
        y_p = _st.enter_context(tc.tile_pool(name="yp", bufs=3))
        dr_p = _st.enter_context(tc.tile_pool(name="dr", bufs=8, space="DRAM"))
        ps_big = _st.enter_context(
            tc.tile_pool(name="ps_big", bufs=3, space="PSUM"))
        ps_s = _st.enter_context(
            tc.tile_pool(name="ps_s", bufs=1, space="PSUM"))
        ps_pv = _st.enter_context(
            tc.tile_pool(name="ps_pv", bufs=int(os.environ.get("K_PV", "2")), space="PSUM"))
        ps_bc = None if DBF else _st.enter_context(
            tc.tile_pool(name="ps_bc", bufs=1, space="PSUM"))
        if True:
            # ---- DMAs: SP queue feeds the mix (gT, packed x) and late
            # weights; ACT queue feeds the k path (wk, xT).  Each HWDGE
            # queue issues serially (~600ns per DMA) so order = first use.
            def load_w(d, nm, eng):
                t = const_p.tile([128, 6 * DIM], bf16, name=nm)
                eng.dma_start(out=t, in_=d[:, :])
                return t

            # mix inputs first, paired across both HWDGE queues (gT+x feed
            # the mix, the only PE work available in the first ~10us);
            # k-path (wk + xT halves) next; the rest in order of first use.
            # token-tile halves loaded separately: no padding bytes.
            xt0 = [const_p.tile([128, 2 * DIM], bf16, name=f"xt0_{bp}")
                   for bp in range(4)]
            xt1 = const_p.tile([128, B_PER_CORE * DIM], bf16, name="xt1")
            xt1v = xt1.rearrange("p (b c) -> p b c", b=B_PER_CORE)
            g_pk = const_p.tile([128, NT2], bf16, name="g_pk")
            nc.scalar.dma_start(out=g_pk, in_=gT_d[:, :])
            nc.sync.dma_start(
                out=xt1v[0:68],
                in_=x_d[:, 128:196, :].rearrange("b p c -> p b c"))
            for bp in range(4):
                (nc.scalar if bp % 2 == 0 else nc.sync).dma_start(
                    out=xt0[bp].rearrange("p (b c) -> p b c", b=2),
                    in_=x_d[2 * bp:2 * bp + 2, 0:128, :]
                    .rearrange("b p c -> p b c"))

            def load_w8(d, nm, eng):
                # [2, 128, 6*DIM]: hi/lo fp8 halves of a 64x-scaled weight
                t = const_p.tile([128, 2 * 6 * DIM], fp8, name=nm)
                eng.dma_start(out=t.rearrange("p (h c) -> p h c", h=2),
                              in_=d[:, :, :].rearrange("h p c -> p h c"))
                return t

            # x8/dx8: [128, kt, NTB] fp8 (DoubleRow pairs slice dim 1)
            x8_sb = big_p.tile([128, 6 * NTB], fp8, name="x8")
            dx8_sb = big_p.tile([128, 6 * NTB], fp8, name="dx8")
            x8v = x8_sb.rearrange("p (k c) -> p k c", k=6)
            dx8v = dx8_sb.rearrange("p (k c) -> p k c", k=6)
            nc.scalar.dma_start(
                out=x8v, in_=x8_d[:, :, :].rearrange("k p c -> p k c"))
            nc.sync.dma_start(
                out=dx8v, in_=dx8_d[:, :, :].rearrange("k p c -> p k c"))
            wk_sb = load_w8(wk_d, "wk", nc.sync)
            wq_sb = load_w8(wq_d, "wq", nc.scalar)
            wv_sb = load_w8(wv_d, "wv", nc.sync)
            wp_sb = load_w(wp_d, "wp", nc.sync)
            bias_sb = const_p.tile([128, DIM], f32, name="bias")
            nc.sync.dma_start(out=bias_sb,
                              in_=bias_d[None, :].broadcast_to([128, DIM]))

            def wsl(w, kt, c0, c1):
                return w[:, kt * DIM + c0:kt * DIM + c1]

            ones64 = const_p.tile([128, 64], bf16, name="ones64")
            nc.vector.memset(ones64, 1.0)

            def warmup(n):
                # dependency-free matmuls: keep the PE clock ramp warm while
                # the first DMAs land (output never read)
                for i in range(n):
                    ps = ps_bc.tile([128, NT2], f32, tag="bc2", name="wu")
                    nc.tensor.matmul(ps[0:64, :], ones64, ones64[:, 0:1]
                                     .broadcast_to([128, NT2]) if False else
                                     ones64[:, 0:1], start=True, stop=True)

            xg8_sb = big_p.tile([128, 6 * NTB], fp8, name="xg8")
            dxg8_sb = big_p.tile([128, 6 * NTB], fp8, name="dxg8")
            xg8v = xg8_sb.rearrange("p (k c) -> p k c", k=6)
            dxg8v = dxg8_sb.rearrange("p (k c) -> p k c", k=6)
            qT_sb = [big_p.tile([128, NTB], bf16, name=f"qT{k}")
                     for k in range(6)]
            kT_sb = [big_p.tile([128, NTB], bf16, name=f"kT{k}")
                     for k in range(6)]
            v_sb = [
                [big_p.tile([128, HEADS * VW], bf16, name=f"v{b}_{ti}")
                 for ti in range(2)]
                for b in range(B_PER_CORE)
            ]
            for b in range(B_PER_CORE):
                for ti in range(2):
                    ones_cols = v_sb[b][ti].rearrange(
                        "p (h e) -> p h e", e=VW)[:, :, 64:65]
                    nc.vector.memset(ones_cols, 1.0)

            # o^T in 3 groups: batches 4-7 / 1-3 / 0.  Attention runs b7
            # first, so group C (b4-7) unblocks its projection mid-pipeline
            # and the tail is only batch 0's 2 windows (deps are
            # tile-granular; a proj stationary AP cannot span group tiles).
            _ogv = 3
            OG = {5: [(0, 1), (1, 2), (2, 4), (4, 6), (6, 8)],
                  4: [(0, 1), (1, 2), (2, 4), (4, 8)],
                  3: [(0, 1), (1, 4), (4, 8)],
                  2: [(0, 4), (4, 8)],
                  6: [(0, 2), (2, 4), (4, 8)]}[_ogv]  # group -> batches
            o_sb = [[big_p.tile([128, (hi - lo) * N_TOK], bf16,
                                name=f"o{k}_{g}")
                     for g, (lo, hi) in enumerate(OG)] for k in range(6)]

            def ogrp(b):
                for g, (lo, hi) in enumerate(OG):
                    if lo <= b < hi:
                        return g, (b - lo) * N_TOK
                raise AssertionError

            # proj windows per group: (g, local w0, wsz); global 128-aligned
            PW = []
            for g, (lo, hi) in enumerate(OG):
                t0g, t1g = lo * N_TOK, hi * N_TOK
                bounds = [t0g] + [t for t in range(0, NTB, 128)
                                  if t0g < t < t1g] + [t1g]
                for a, bnd in zip(bounds, bounds[1:]):
                    PW.append((g, a - t0g, bnd - a))

            # ---- emission helpers ----
            MIX_POOLS = None  # set after pools exist

            def mix_chunk(bp, mt, _eng=None):
                # two batches (2bp, 2bp+1) per psum bank; round-robin all
                # four psum pools so copy latency never gates the PE
                pool, tag = MIX_POOLS[(bp * 6 + mt) % len(MIX_POOLS)]
                ps = pool.tile([128, NT2], f32, tag=tag, name="psm")
                for i in range(2):
                    b = 2 * bp + i
                    for ti, (t0, tsz) in enumerate(TOK_TILES):
                        if ti == 0:
                            st = xt0[bp][:tsz, i * DIM + mt * 128:
                                         i * DIM + (mt + 1) * 128]
                        else:
                            st = xt1v[:tsz, b, mt * 128:(mt + 1) * 128]
                        nc.tensor.matmul(
                            ps[:, i * N_TOK:(i + 1) * N_TOK], st,
                            g_pk[:tsz, ti * N_TOK:(ti + 1) * N_TOK],
                            start=(ti == 0), stop=(ti == 1),
                        )
                cs = slice(bp * NT2, (bp + 1) * NT2)
                nc.scalar.activation(xg8v[:, mt, cs], ps, AF.Copy)
                nc.vector.tensor_sub(dxg8v[:, mt, cs], ps, xg8v[:, mt, cs])

            DR = mybir.MatmulPerfMode.DoubleRow

            def qk_chunk(which, mt, nt):
                if which == "q":
                    dst, w, xhi, xlo = qT_sb, wq_sb, xg8v, dxg8v
                    cbase = nt * NT2
                else:
                    dst, w = kT_sb, wk_sb
                    xhi, xlo = x8v, dx8v
                    cbase = nt * NT2
                wv_ = w.rearrange("p (h k m) -> p h k m", h=2, k=6)
                ps = ps_big.tile([128, NT2], f32, tag="psA", name="ps")
                ms = slice(mt * 128, (mt + 1) * 128)
                nmm = 0
                for half in range(2):
                    c0 = cbase + half * N_TOK
                    cs = slice(c0, c0 + N_TOK)
                    for t in range(3):
                        kp = slice(2 * t, 2 * t + 2)
                        for whalf, xsrc in ((0, xhi), (1, xhi), (0, xlo)):
                            nc.tensor.matmul(
                                ps[:, half * N_TOK:(half + 1) * N_TOK],
                                wv_[:, whalf, kp, ms], xsrc[:, kp, cs],
                                start=(nmm == 0), stop=(nmm == 17),
                                perf_mode=DR,
                            )
                            nmm += 1
                par = 0
                sel = ((mt + nt) % 2 if par == 0 else
                       (mt % 2 if par == 1 else nt % 2))
                if not SCHED["qk_alt"] or sel:
                    nc.scalar.activation(
                        dst[mt][:, nt * NT2:(nt + 1) * NT2], ps, AF.Copy)
                else:
                    nc.vector.tensor_copy(
                        dst[mt][:, nt * NT2:(nt + 1) * NT2], ps)

            def v_chunk(b, ti, nt):
                t0, tsz = TOK_TILES[ti]
                veng = 0
                eng = (b * 4 + ti * 2 + nt) % 2 if veng == 0 else veng - 1
                wvv = wv_sb.rearrange("p (h k m) -> p h k m", h=2, k=6)
                ps = ps_big.tile([128, NT2], f32, tag="psA", name="ps")
                xa, dxa = x8v, dx8v
                xs = slice(b * N_TOK + t0, b * N_TOK + t0 + tsz)
                nmm = 0
                for half in range(2):
                    m0 = nt * 384 + half * 192
                    ms = slice(m0, m0 + 192)
                    for t in range(3):
                        kp = slice(2 * t, 2 * t + 2)
                        for whalf, xsrc in ((0, xa), (0, dxa), (1, xa)):
                            nc.tensor.matmul(
                                ps[:tsz, half * 192:half * 192 + 192],
                                xsrc[:, kp, xs], wvv[:, whalf, kp, ms],
                                start=(nmm == 0), stop=(nmm == 17),
                                perf_mode=DR,
                            )
                            nmm += 1
                dst = v_sb[b][ti].rearrange(
                    "p (h e) -> p h e", e=VW)[:tsz, 6 * nt:6 * nt + 6, 0:64]
                src = ps[:tsz, :384].rearrange("p (h e) -> p h e", e=64)
                if eng:
                    nc.scalar.activation(dst, src, AF.Copy)
                else:
                    nc.vector.tensor_copy(dst, src)

            ystash = {}

            def proj_chunk(g, w0, wsz, nt):
                ps = ps_big.tile([128, NT2], f32, tag="psA", name="ps")
                for kt in range(6):
                    nc.tensor.matmul(
                        ps[:wsz, :384],
                        o_sb[kt][g][:, w0:w0 + wsz],
                        wsl(wp_sb, kt, nt * 384, (nt + 1) * 384),
                        start=(kt == 0), stop=(kt == 5),
                    )
                if nt == 0:
                    y_sb = y_p.tile([128, DIM], f32, tag="y", name="y_sb")
                    ystash[(g, w0)] = y_sb
                else:
                    y_sb = ystash.pop((g, w0))
                nc.vector.tensor_add(
                    y_sb[:wsz, nt * 384:(nt + 1) * 384], ps[:wsz, :384],
                    bias_sb[:wsz, nt * 384:(nt + 1) * 384])
                if nt == 1:
                    a0 = OG[g][0] * N_TOK + w0
                    nc.sync.dma_start(out=out_d[a0:a0 + wsz, :],
                                      in_=y_sb[:wsz])

            # ---- attention stages (3-stage software pipeline) ----
            stash = {}

            def attn_s_exp(b, p):
                # S for BOTH heads in one 2-bank psum tile (cols 0:392 and
                # 512:904 - each matmul group stays inside one bank), then a
                # single strided exp covers both.
                c0 = b * N_TOK
                s_ps = ps_s.tile([128, 1024], f32, tag="s", name="s")
                for hi in range(2):
                    hb = hi * 64
                    for ti, (t0, tsz) in enumerate(TOK_TILES):
                        nc.tensor.matmul(
                            s_ps[:tsz,
                                 hi * 512 + ti * N_TOK:
                                 hi * 512 + (ti + 1) * N_TOK],
                            kT_sb[p][hb:hb + 64, c0 + t0:c0 + t0 + tsz],
                            qT_sb[p][hb:hb + 64, c0:c0 + N_TOK],
                            start=True, stop=True,
                        )
                pT = cp_p.tile([128, 2 * NT2], bf16, tag="pT", name="pT")
                sv = s_ps.rearrange("q (h c) -> q h c", h=2)[:, :, 0:NT2]
                pv_ = pT.rearrange("q (h c) -> q h c", h=2)
                nc.scalar.activation(pv_, sv, AF.Exp, scale=1.0 / S_SCALE)
                stash[(b, p, "pT")] = pT

            def attn_pv(b, p):
                pT = stash.pop((b, p, "pT"))
                pv = ps_pv.tile([128, NT2], f32, tag="pv", name="pv")
                for hi in range(2):
                    h = 2 * p + hi
                    for ti, (t0, tsz) in enumerate(TOK_TILES):
                        nc.tensor.matmul(
                            pv[0:VW, hi * N_TOK:(hi + 1) * N_TOK],
                            v_sb[b][ti][:tsz, h * VW:(h + 1) * VW],
                            pT[:tsz,
                               hi * NT2 + ti * N_TOK:
                               hi * NT2 + (ti + 1) * N_TOK],
                            start=(ti == 0), stop=(ti == 1),
                        )
                if SCHED["dram_bcast"]:
                    stash[(b, p, "pv")] = (pv, None)
                    return
                if 0:
                    stash[(b, p, "pv")] = (pv, None)
                    return
                srow = cp_p.tile([128, NT2], bf16, tag="srow", name="srow")
                if not SCHED["sums_alt"] or (b + p) % 2:
                    nc.scalar.activation(srow[64:65, :], pv[64:65, :], AF.Copy)
                else:
                    nc.vector.tensor_copy(srow[64:65, :], pv[64:65, :])
                stash[(b, p, "pv")] = (pv, srow)

            def attn_norm_a(b, p):
                # reciprocal of the sums row straight from psum, spill to DRAM
                pv, _ = stash[(b, p, "pv")]
                rrow = cp_p.tile([128, NT2], bf16, tag="rrow", name="rrow")
                with nc.allow_low_precision(reason="softmax recip bf16"):
                    nc.vector.reciprocal(rrow[64:65, :], pv[64:65, :])
                d = dr_p.tile([1, NT2], bf16, name="rd")
                nc.sync.dma_start(out=d, in_=rrow[64:65, :])
                stash[(b, p, "rd")] = d

            def attn_norm_b(b, p):
                # DRAM -> SBUF partition-broadcast of the reciprocal row
                d = stash.pop((b, p, "rd"))
                rbc = cp_p.tile([128, NT2], bf16, tag="rbc", name="rbc")
                nc.sync.dma_start(
                    out=rbc[0:64, :],
                    in_=d[0, :][None, :].broadcast_to([64, NT2]))
                stash[(b, p, "rbc")] = rbc

            def attn_norm_c(b, p):
                g, c0 = ogrp(b)
                pv, _ = stash.pop((b, p, "pv"))
                rbc = stash.pop((b, p, "rbc"))
                nc.vector.tensor_mul(o_sb[p][g][0:64, c0:c0 + N_TOK],
                                     pv[0:64, 0:N_TOK], rbc[0:64, 0:N_TOK])
                nc.vector.tensor_mul(o_sb[p][g][64:128, c0:c0 + N_TOK],
                                     pv[0:64, N_TOK:NT2], rbc[0:64, N_TOK:NT2])

            def attn_norm(b, p):
                g, c0 = ogrp(b)
                pv, srow = stash.pop((b, p, "pv"))
                if srow is None:
                    srow = cp_p.tile([128, NT2], bf16, tag="srow", name="srow")
                    nc.scalar.activation(srow[64:65, :], pv[64:65, :], AF.Copy)
                bc = ps_bc.tile([128, NT2], f32, tag="bc2", name="bc")
                nc.tensor.matmul(
                    bc[0:64, :], ones64[64:65, :], srow[64:65, :],
                    start=True, stop=True, tile_position=(64, 0),
                )
                rcp = cp_p.tile([128, NT2], bf16, tag="rcp", name="rcp")
                with nc.allow_low_precision(reason="softmax recip bf16"):
                    nc.vector.reciprocal(rcp[0:64, :], bc[0:64, :])
                nc.vector.tensor_mul(o_sb[p][g][0:64, c0:c0 + N_TOK],
                                     pv[0:64, 0:N_TOK], rcp[0:64, 0:N_TOK])
                nc.vector.tensor_mul(o_sb[p][g][64:128, c0:c0 + N_TOK],
                                     pv[0:64, N_TOK:NT2], rcp[0:64, N_TOK:NT2])

            # ---- schedule ----
            MIX_POOLS = [(ps_s, "s"), (ps_pv, "pv"), (ps_big, "psA")]
            if ps_bc is not None:
                MIX_POOLS.insert(0, (ps_bc, "bc2"))
            # mix bp0-1 first (x0-x3 land early), then k 1:1 with the
            # remaining mix chunks (independent), then q.
            mixes = [(bp, mt) for bp in range(4) for mt in range(6)]
            nup = 24
            for bp, mt in mixes[:nup]:
                mix_chunk(bp, mt, "")
            mix_rest = mixes[nup:]
            if 0:
                korder = [(mt, nt) for mt in range(6) for nt in range(4)]
            else:
                korder = [(mt, nt) for nt in range(4) for mt in range(6)]
            for i, (mt, nt) in enumerate(korder):
                qk_chunk("k", mt, nt)
                if i < len(mix_rest):
                    mix_chunk(*mix_rest[i], "")
            if int(os.environ.get("K_QORD", "0")):
                for mt in range(6):
                    for nt in range(4):
                        qk_chunk("q", mt, nt)
            else:
                for nt in range(4):
                    for mt in range(6):
                        qk_chunk("q", mt, nt)

            # attention-window dense stream: v batches 7..0, proj on
            # group eligibility
            dense = []
            vpos = {}
            vlate = []
            for vb in range(7, -1, -1):
                tgt = dense if vb >= int(os.environ.get("K_VHOLD", "3")) else vlate
                for ti in range(2):
                    for nt in range(2):
                        tgt.append(("v", vb, ti, nt))
                vpos[vb] = (len(dense) - 1 if vb >= 2
                            else 24 + 2 + len(vlate) + 2)

            # attention unit order: descending batch; each o-group's proj
            # unlocks when its lowest batch (last in this order) is normed
            units = [(b, p) for b in range(7, -1, -1) for p in range(6)]

            MARGIN = int(os.environ.get("K_MARGIN", "0"))
            proj_after = {}  # last batch of group (in unit order) -> chunks
            for g, (lo, hi) in enumerate(OG):
                gws = [pw for pw in PW if pw[0] == g]
                if g != 0 or 0:
                    gws = gws[::-1]
                proj_after[lo] = [  # b7-first => smallest batch finishes last
                    ("p",) + w + (nt,)
                    for w in gws
                    for nt in range(2)]

            emitted = 0
            norm_count = {b: 0 for b in range(B_PER_CORE)}

            def emit_dense(n):
                nonlocal_emitted = 0
                while nonlocal_emitted < n and dense:
                    c = dense.pop(0)
                    if c[0] == "v":
                        v_chunk(*c[1:])
                    elif c[0] in ("k", "q"):
                        qk_chunk(c[0], c[1], c[2])
                    else:
                        proj_chunk(*c[1:])
                    nonlocal_emitted += 1
                return nonlocal_emitted

            def unit_ready(i):
                b, p = units[i]
                if b < int(os.environ.get("K_VHOLD", "3")):
                    # late-released v: gated by the release point
                    return i >= 27 and emitted >= vpos[b]
                return emitted >= vpos[b] + MARGIN

            i = 0
            n_units = len(units)
            credit = 0.0
            # dense chunks that will EVER be available during attention:
            total_dense_all = len(dense) + len(vlate) + sum(
                len(v) for v in proj_after.values())
            pace = total_dense_all / float(n_units) * float(os.environ.get("K_PACE", "1.0"))
            DB = SCHED["dram_bcast"]
            n_stages = 5 if DB else 3
            while i < n_units + n_stages - 1:
                if i < n_units and not unit_ready(i):
                    got = emit_dense(1)
                    emitted += got
                    if got == 0:
                        # dense exhausted but unit not "ready": emit anyway
                        pass
                    else:
                        continue
                if i < n_units:
                    attn_s_exp(*units[i])
                if 1 <= i < n_units + 1:
                    attn_pv(*units[i - 1])
                if i == 22 and vlate:
                    dense.extend(vlate)
                    vlate = []
                if DB:
                    if 2 <= i < n_units + 2:
                        attn_norm_a(*units[i - 2])
                    if 3 <= i < n_units + 3:
                        attn_norm_b(*units[i - 3])
                    if 4 <= i:
                        b, p = units[i - 4]
                        attn_norm_c(b, p)
                        norm_count[b] += 1
                        if norm_count[b] == 6 and b in proj_after:
                            dense.extend(proj_after.pop(b))
                elif 2 <= i:
                    b, p = units[i - 2]
                    attn_norm(b, p)
                    norm_count[b] += 1
                    if norm_count[b] == 6 and b in proj_after:
                        dense.extend(proj_after.pop(b))
                credit += pace
                take = int(credit)
                credit -= take
                emitted += emit_dense(take)
                i += 1
            while dense:
                emitted += emit_dense(1)

    return nc


_CACHED_NC = None


def kernel(x, w_qkv, w_proj, b_proj, factors):
    global LAST_EXEC_NS, LAST_TRACE, _CACHED_NC
    from concourse.bass_utils import run_bass_kernel_spmd

    E4M3 = ml_dtypes.float8_e4m3fn
    WS, XG_SCALE = 64.0, 16.0
    factors = np.asarray(factors, dtype=np.float32)
    scale = HEAD_DIM ** -0.5
    G_s = _grid_g(factors) * scale
    gT_f = G_s.T * XG_SCALE  # [196, 196], fp8-range scaled
    gT = np.zeros((128, NT2), dtype=np.float32)
    gT[:, 0:N_TOK] = gT_f[0:128]
    gT[0:68, N_TOK:NT2] = gT_f[128:196]
    gT = gT.astype(BF16)

    w_qkv = np.asarray(w_qkv, dtype=np.float32)

    def il(wT, d=BF16):
        # [768, 768] feature-major -> [128, 6*768] kt-interleaved
        return np.ascontiguousarray(
            wT.reshape(6, 128, DIM).transpose(1, 0, 2).reshape(128, 6 * DIM)
        ).astype(d)

    def split8(wT):
        # 64x-scaled fp8 hi/lo halves, kt-interleaved: [2, 128, 6*DIM]
        ws = il(wT * WS, np.float32)
        hi = ws.astype(E4M3)
        lo = (ws - hi.astype(np.float32)).astype(E4M3)
        return np.stack([hi, lo])

    in_common = {
        "gT": gT,
        "wq8": split8(w_qkv[0:DIM, :].T),
        "wk8": split8(w_qkv[DIM:2 * DIM, :].T),
        "wv8": split8(w_qkv[2 * DIM:3 * DIM, :].T),
        "wpT": il(np.asarray(w_proj, dtype=np.float32).T / WS),
        "bias": np.asarray(b_proj, dtype=np.float32),
    }
    x = np.asarray(x, dtype=np.float32).astype(BF16)
    in_maps = []
    for c in range(N_CORES):
        xc = x[c * B_PER_CORE:(c + 1) * B_PER_CORE]  # [8, 196, 768]
        xT = np.ascontiguousarray(
            xc.transpose(2, 0, 1).reshape(6, 128, NTB)).astype(np.float32)
        x8 = xT.astype(E4M3)
        dx8 = (xT - x8.astype(np.float32)).astype(E4M3)
        in_maps.append({"x": np.ascontiguousarray(xc), "x8T": x8,
                        "dx8T": dx8, **in_common})

    if _CACHED_NC is None:
        _CACHED_NC = _build_bass()
    nc = _CACHED_NC

    trace = bool(int(os.environ.get("KERNEL_TRACE", "0")))
    res = run_bass_kernel_spmd(nc, in_maps, core_ids=list(range(N_CORES)),
                               trace=trace)
    LAST_EXEC_NS = res.exec_time_ns
    if res.instructions_and_trace is not None:
        LAST_TRACE = res.instructions_and_trace[1]
    out = np.concatenate(
        [res.results[c]["out"].reshape(B_PER_CORE, N_TOK, DIM)
         for c in range(N_CORES)], axis=0)
    return out.astype(np.float32)
